# revision 19
# baseline (speedup 1.0000x reference)
"""Trainium2 Bass kernel for nn_LIMADNN2_42013370090068 (dense_mlp).

Reference semantics: out depends only on x[:, 0, :] — the `state.add(...)`
neighbor loop in the torch module is not in-place, so the 65-neighbor
dimension is dead. force_prev = x[:, 0, 6:9] is a pure slice.

  q   = x[:, 0, :]                 # [B, 12]
  h   = relu(q @ W1 + b1)          # [B, 16]
  blk = relu(h @ W2 + b2)          # [B, 8]
  out = (blk @ Ws + bs) @ Wo + bo  # [B, 3]   (no relu between -> folded)

Device strategy (pure data parallel, 8 cores, batch-sharded):
  * Host slices q (12.6 MB of the 818 MB input), computes force_prev, and
    folds Ws/Wo into one [8,3] matrix. All matmul operands bf16 (fp32
    PSUM): 1 cyc/col on the PE, half the HBM bytes; end-to-end rel err
    ~7e-3 vs the 2e-2 gate.
  * Features-on-partitions, 8 batch-chunks block-diagonal per matmul,
    N=1024 moving columns (bf16 max) to minimize instruction count —
    per-matmul sequencer cost (ldweights+matmul ~340 ns) dominates over
    column streaming for this size.
  * L2 outputs of two consecutive supergroups land in one [128,1024]
    PSUM tile at partition offsets 0/64, halving relu2 columns; L3 uses
    16-chunk block-diag [128,48] per pair, pairs at offsets 0/64 of one
    [112,1024] PSUM tile (rows 48-63/112-127 dead), so two copy+bias ops
    and two DMAs drain the whole core's output.
  * PE HAM warm-up: dummy matmuls on the weight tile while input DMAs
    stream; ACT activation-table preload via a dummy relu at t=0.
  * f32 biases ride bit-packed in the bf16 weight tile (bitcast APs) and
    are folded into the PSUM->SBUF ops; out bias bso added on-device.
"""

import numpy as np
import ml_dtypes

B = 262144
F = 12
N_CORES = 8
BPC = B // N_CORES          # 32768 atoms per core
CHUNKS = 8                  # batch chunks packed on PE partitions (L1/L2)
SG = 4                      # supergroups per core
SGW = 1024                  # moving columns per supergroup matmul
FREE = SG * SGW             # 4096 input columns per core
WCOLS = 246                 # packed weight tensor columns (bf16; f32 biases
                            # bit-packed as bf16 pairs at cols 240-245)
N_WARMUP = 8                # PE warm-up matmuls (N=240 each)

BF16 = ml_dtypes.bfloat16


def _build_nc():
    import concourse.tile as tile
    from concourse import bacc, mybir

    f32 = mybir.dt.float32
    bf16 = mybir.dt.bfloat16

    nc = bacc.Bacc("TRN2", target_bir_lowering=False, debug=False,
                   num_devices=N_CORES)

    xin = nc.dram_tensor("xin", [CHUNKS * F, FREE], bf16, kind="ExternalInput")
    wpack = nc.dram_tensor("wpack", [128, WCOLS], bf16, kind="ExternalInput")
    # rows: 64*pair + 24*sp + 3*chunk + f; rows 48-63/112-127 dead padding
    out = nc.dram_tensor("out", [112, SGW], bf16, kind="ExternalOutput")

    Relu = mybir.ActivationFunctionType.Relu
    add, vmax = mybir.AluOpType.add, mybir.AluOpType.max

    def mm(ps_ap, lhsT_ap, rhs_ap):
        nc.tensor.matmul(ps_ap, lhsT_ap, rhs_ap, start=True, stop=True)

    with tile.TileContext(nc) as tc:
        with (
            tc.tile_pool(name="const", bufs=1) as cpool,
            tc.tile_pool(name="dmy", bufs=1) as dpool,
            tc.tile_pool(name="xt", bufs=4) as xpool,
            tc.tile_pool(name="h", bufs=2) as hpool,
            tc.tile_pool(name="blk", bufs=2) as bpool,
            tc.tile_pool(name="osb", bufs=2) as opool,
            tc.tile_pool(name="ps1", bufs=2, space="PSUM") as ps1pool,
            tc.tile_pool(name="ps2", bufs=1, space="PSUM") as ps2pool,
            tc.tile_pool(name="ps3", bufs=1, space="PSUM") as ps3pool,
        ):
            # ACT activation-table preload: dummy relu on a memset tile so
            # the auto-inserted LoadActFuncSet (~1.3 us) runs at t=0,
            # concurrent with the input DMAs, not before the first real relu.
            dmy = dpool.tile([1, 2], bf16)
            nc.vector.memset(dmy[:], 0.0)
            dmy2 = dpool.tile([1, 2], bf16)
            nc.scalar.activation(dmy2[:], dmy[:], Relu)

            # Weight DMA issued from the ACT sequencer (HWDGE) so its config
            # overlaps SP issuing the first input chunk.
            wsb = cpool.tile([128, WCOLS], bf16)
            nc.scalar.dma_start(wsb[:], wpack[:])
            w1 = wsb[0:96, 0:128]
            w2 = wsb[0:128, 128:192]
            w3 = wsb[0:128, 192:240]
            b1 = wsb[0:128, 240:242].bitcast(f32)
            b2 = wsb[0:128, 242:244].bitcast(f32)

            # Keep the PE busy (HAM warm-up) while input chunks stream in.
            # Warm-up matmuls write into the ps3 tile (no spare PSUM bank);
            # the real mm3s overwrite rows 0-47/64-111 later and the dead
            # rows are dropped on host.
            ps3 = ps3pool.tile([128, SGW], f32)
            for _ in range(N_WARMUP):
                mm(ps3[:, 0:240], wsb[0:128, 0:128], wsb[0:128, 0:240])

            ps2 = None
            for s in range(SG):
                xt = xpool.tile([CHUNKS * F, SGW], bf16)
                nc.sync.dma_start(xt[:], xin[:, SGW * s:SGW * (s + 1)])

                ps1 = ps1pool.tile([128, SGW], f32)
                mm(ps1[:], w1, xt[:])
                h = hpool.tile([128, SGW], bf16)
                nc.scalar.activation(h[:], ps1[:], Relu, bias=b1)

                sp = s % 2
                if sp == 0:
                    ps2 = ps2pool.tile([128, SGW], f32)
                mm(ps2[64 * sp:64 * sp + 64, :], w2, h[:])

                if sp == 1:
                    p = s // 2
                    blk = bpool.tile([128, SGW], bf16)
                    nc.vector.tensor_scalar(blk[:], ps2[:], b2, 0.0, add, vmax)
                    mm(ps3[64 * p:64 * p + 48, :], w3, blk[:])
                    # Drain this pair immediately: copy+bias then DMA out
                    # (issued from ACT's sequencer: SP stays input-only and
                    # pair 0's output overlaps the rest of the pipeline).
                    rows = slice(64 * p, 64 * p + 48)
                    bso_p = wsb[64 * p:64 * p + 48, 244:246].bitcast(f32)
                    osb = opool.tile([48, SGW], bf16)
                    nc.vector.tensor_scalar(osb[:], ps3[rows, :], bso_p,
                                            None, add)
                    nc.scalar.dma_start(out[rows, :], osb[:])

    nc.finalize()
    return nc


def _host_prep(x, W1, b1, W2, b2, Ws, bs, Wo, bo):
    x = np.asarray(x)
    W1 = np.asarray(W1, dtype=np.float32)
    b1 = np.asarray(b1, dtype=np.float32)
    W2 = np.asarray(W2, dtype=np.float32)
    b2 = np.asarray(b2, dtype=np.float32)
    Ws = np.asarray(Ws, dtype=np.float32)
    bs = np.asarray(bs, dtype=np.float32)
    Wo = np.asarray(Wo, dtype=np.float32)
    bo = np.asarray(bo, dtype=np.float32)

    q = np.ascontiguousarray(x[:, 0, :], dtype=np.float32)       # [B, 12]
    force_prev = np.ascontiguousarray(x[:, 0, 6:9], dtype=np.float32)

    # Fold the two linear layers that have no nonlinearity between them.
    Wso = (Ws.astype(np.float64) @ Wo.astype(np.float64)).astype(np.float32)
    bso = (bs.astype(np.float64) @ Wo.astype(np.float64)
           + bo.astype(np.float64)).astype(np.float32)

    wts = np.zeros((128, 240), np.float32)
    for c in range(CHUNKS):
        wts[c * 12:(c + 1) * 12, c * 16:(c + 1) * 16] = W1
        wts[c * 16:(c + 1) * 16, 128 + c * 8:128 + (c + 1) * 8] = W2
    for k in range(16):
        wts[k * 8:(k + 1) * 8, 192 + k * 3:192 + (k + 1) * 3] = Wso
    p = np.arange(128)
    biases = np.zeros((128, 3), np.float32)
    biases[:, 0] = b1[p % 16]
    biases[:, 1] = b2[p % 8]
    biases[0:48, 2] = bso[p[0:48] % 3]
    biases[64:112, 2] = bso[p[0:48] % 3]
    wpack = np.zeros((128, WCOLS), BF16)
    wpack[:, 0:240] = wts.astype(BF16)
    # f32 biases bit-packed as bf16 pairs (little-endian), bitcast on device
    wpack[:, 240:246] = biases.view(np.uint16).view(BF16)

    qb = q.astype(BF16)
    in_maps = []
    for c in range(N_CORES):
        qc = qb[c * BPC:(c + 1) * BPC]
        # atom n = s*8192 + ch*1024 + j  ->  partition 12*ch+f, col s*1024+j
        Ac = np.ascontiguousarray(
            qc.reshape(SG, CHUNKS, SGW, F)
              .transpose(1, 3, 0, 2).reshape(CHUNKS * F, FREE))
        in_maps.append({"xin": Ac, "wpack": wpack})
    return in_maps, force_prev


def _host_gather(results):
    out = np.empty((B, 3), np.float32)
    for c in range(N_CORES):
        Oc = np.asarray(results[c]["out"]).astype(np.float32)    # [112, 1024]
        Oc = np.concatenate([Oc[0:48], Oc[64:112]])              # drop pad
        # row = 48*p + 24*sp + 3*ch + f; col j; n = (2p+sp)*8192 + ch*1024 + j
        oc = (Oc.reshape(2, 2, 8, 3, SGW)
                .transpose(0, 1, 2, 4, 3).reshape(BPC, 3))
        out[c * BPC:(c + 1) * BPC] = oc
    return out


_LAST_RES = None  # BassKernelResults of the most recent run (for test harness)


def kernel(x, W1, b1, W2, b2, Ws, bs, Wo, bo):
    global _LAST_RES
    from concourse.bass_utils import run_bass_kernel_spmd

    in_maps, force_prev = _host_prep(x, W1, b1, W2, b2, Ws, bs, Wo, bo)
    nc = _build_nc()
    res = run_bass_kernel_spmd(nc, in_maps, core_ids=list(range(N_CORES)))
    _LAST_RES = res
    out = _host_gather(res.results)
    return (out, force_prev)


# revision 20
# speedup vs baseline: 1.1584x; 1.1584x over previous
"""Trainium2 Bass kernel for nn_LIMADNN2_42013370090068 (dense_mlp).

Reference semantics: out depends only on x[:, 0, :] — the `state.add(...)`
neighbor loop in the torch module is not in-place, so the 65-neighbor
dimension is dead. force_prev = x[:, 0, 6:9] is a pure slice.

  q   = x[:, 0, :]                 # [B, 12]
  h   = relu(q @ W1 + b1)          # [B, 16]
  blk = relu(h @ W2 + b2)          # [B, 8]
  out = (blk @ Ws + bs) @ Wo + bo  # [B, 3]   (no relu between -> folded)

Device strategy (pure data parallel, 8 cores, batch-sharded):
  * Host slices q (12.6 MB of the 818 MB input), computes force_prev, and
    folds Ws/Wo into one [8,3] matrix. All matmul operands bf16 (fp32
    PSUM): 1 cyc/col on the PE, half the HBM bytes; end-to-end rel err
    ~7e-3 vs the 2e-2 gate.
  * Features-on-partitions, 8 batch-chunks block-diagonal per matmul,
    N=1024 moving columns (bf16 max) to minimize instruction count —
    per-matmul sequencer cost (ldweights+matmul ~340 ns) dominates over
    column streaming for this size.
  * L2 outputs of two consecutive supergroups land in one [128,1024]
    PSUM tile at partition offsets 0/64, halving relu2 columns; L3 uses
    16-chunk block-diag [128,48] per pair, pairs at offsets 0/64 of one
    [112,1024] PSUM tile (rows 48-63/112-127 dead), so two copy+bias ops
    and two DMAs drain the whole core's output.
  * PE HAM warm-up: dummy matmuls on the weight tile while input DMAs
    stream; ACT activation-table preload via a dummy relu at t=0.
  * f32 biases ride bit-packed in the bf16 weight tile (bitcast APs) and
    are folded into the PSUM->SBUF ops; out bias bso added on-device.
"""

import numpy as np
import ml_dtypes

B = 262144
F = 12
N_CORES = 8
BPC = B // N_CORES          # 32768 atoms per core
CHUNKS = 8                  # batch chunks packed on PE partitions (L1/L2)
SG = 4                      # supergroups per core
SGW = 1024                  # moving columns per supergroup matmul
FREE = SG * SGW             # 4096 input columns per core
WCOLS = 246                 # packed weight tensor columns (bf16; f32 biases
                            # bit-packed as bf16 pairs at cols 240-245)
N_WARMUP = 8                # PE warm-up matmuls (N=240 each)

BF16 = ml_dtypes.bfloat16


def _build_nc():
    import concourse.tile as tile
    from concourse import bacc, mybir

    f32 = mybir.dt.float32
    bf16 = mybir.dt.bfloat16

    nc = bacc.Bacc("TRN2", target_bir_lowering=False, debug=False,
                   num_devices=N_CORES)

    xin = nc.dram_tensor("xin", [CHUNKS * F, FREE], bf16, kind="ExternalInput")
    wpack = nc.dram_tensor("wpack", [128, WCOLS], bf16, kind="ExternalInput")
    # rows: 64*pair + 24*sp + 3*chunk + f; rows 48-63/112-127 dead padding
    out = nc.dram_tensor("out", [112, SGW], bf16, kind="ExternalOutput")

    Relu = mybir.ActivationFunctionType.Relu
    add, vmax = mybir.AluOpType.add, mybir.AluOpType.max

    def mm(ps_ap, lhsT_ap, rhs_ap):
        nc.tensor.matmul(ps_ap, lhsT_ap, rhs_ap, start=True, stop=True)

    with tile.TileContext(nc) as tc:
        with (
            tc.tile_pool(name="const", bufs=1) as cpool,
            tc.tile_pool(name="dmy", bufs=1) as dpool,
            tc.tile_pool(name="xt", bufs=4) as xpool,
            tc.tile_pool(name="h", bufs=2) as hpool,
            tc.tile_pool(name="blk", bufs=2) as bpool,
            tc.tile_pool(name="osb", bufs=2) as opool,
            tc.tile_pool(name="ps1", bufs=2, space="PSUM") as ps1pool,
            tc.tile_pool(name="ps2", bufs=1, space="PSUM") as ps2pool,
            tc.tile_pool(name="ps3", bufs=1, space="PSUM") as ps3pool,
        ):
            # ACT activation-table preload: dummy relu on a memset tile so
            # the auto-inserted LoadActFuncSet (~1.3 us) runs at t=0,
            # concurrent with the input DMAs, not before the first real relu.
            dmy = dpool.tile([1, 2], bf16)
            nc.vector.memset(dmy[:], 0.0)
            dmy2 = dpool.tile([1, 2], bf16)
            nc.scalar.activation(dmy2[:], dmy[:], Relu)

            # Weight DMA issued from the ACT sequencer (HWDGE) so its config
            # overlaps SP issuing the first input chunk.
            wsb = cpool.tile([128, WCOLS], bf16)
            nc.scalar.dma_start(wsb[:], wpack[:])
            w1 = wsb[0:96, 0:128]
            w2 = wsb[0:128, 128:192]
            w3 = wsb[0:128, 192:240]
            b1 = wsb[0:128, 240:242].bitcast(f32)
            b2 = wsb[0:128, 242:244].bitcast(f32)

            # No PE warm-up matmuls: the first real matmul lands after the
            # ~3.5 us input-DMA latency, which already satisfies the HAM
            # ramp window; issuing PE work earlier only resets the ramp.
            ps3 = ps3pool.tile([128, SGW], f32)

            ps2 = None
            for s in range(SG):
                xt = xpool.tile([CHUNKS * F, SGW], bf16)
                nc.sync.dma_start(xt[:], xin[:, SGW * s:SGW * (s + 1)])

                ps1 = ps1pool.tile([128, SGW], f32)
                mm(ps1[:], w1, xt[:])
                h = hpool.tile([128, SGW], bf16)
                nc.scalar.activation(h[:], ps1[:], Relu, bias=b1)

                sp = s % 2
                if sp == 0:
                    ps2 = ps2pool.tile([128, SGW], f32)
                mm(ps2[64 * sp:64 * sp + 64, :], w2, h[:])

                if sp == 1:
                    p = s // 2
                    blk = bpool.tile([128, SGW], bf16)
                    nc.vector.tensor_scalar(blk[:], ps2[:], b2, 0.0, add, vmax)
                    mm(ps3[64 * p:64 * p + 48, :], w3, blk[:])
                    # Drain this pair immediately: copy+bias then DMA out
                    # (issued from ACT's sequencer: SP stays input-only and
                    # pair 0's output overlaps the rest of the pipeline).
                    rows = slice(64 * p, 64 * p + 48)
                    bso_p = wsb[64 * p:64 * p + 48, 244:246].bitcast(f32)
                    osb = opool.tile([48, SGW], bf16)
                    nc.vector.tensor_scalar(osb[:], ps3[rows, :], bso_p,
                                            None, add)
                    nc.scalar.dma_start(out[rows, :], osb[:])

    nc.finalize()
    return nc


def _host_prep(x, W1, b1, W2, b2, Ws, bs, Wo, bo):
    x = np.asarray(x)
    W1 = np.asarray(W1, dtype=np.float32)
    b1 = np.asarray(b1, dtype=np.float32)
    W2 = np.asarray(W2, dtype=np.float32)
    b2 = np.asarray(b2, dtype=np.float32)
    Ws = np.asarray(Ws, dtype=np.float32)
    bs = np.asarray(bs, dtype=np.float32)
    Wo = np.asarray(Wo, dtype=np.float32)
    bo = np.asarray(bo, dtype=np.float32)

    q = np.ascontiguousarray(x[:, 0, :], dtype=np.float32)       # [B, 12]
    force_prev = np.ascontiguousarray(x[:, 0, 6:9], dtype=np.float32)

    # Fold the two linear layers that have no nonlinearity between them.
    Wso = (Ws.astype(np.float64) @ Wo.astype(np.float64)).astype(np.float32)
    bso = (bs.astype(np.float64) @ Wo.astype(np.float64)
           + bo.astype(np.float64)).astype(np.float32)

    wts = np.zeros((128, 240), np.float32)
    for c in range(CHUNKS):
        wts[c * 12:(c + 1) * 12, c * 16:(c + 1) * 16] = W1
        wts[c * 16:(c + 1) * 16, 128 + c * 8:128 + (c + 1) * 8] = W2
    for k in range(16):
        wts[k * 8:(k + 1) * 8, 192 + k * 3:192 + (k + 1) * 3] = Wso
    p = np.arange(128)
    biases = np.zeros((128, 3), np.float32)
    biases[:, 0] = b1[p % 16]
    biases[:, 1] = b2[p % 8]
    biases[0:48, 2] = bso[p[0:48] % 3]
    biases[64:112, 2] = bso[p[0:48] % 3]
    wpack = np.zeros((128, WCOLS), BF16)
    wpack[:, 0:240] = wts.astype(BF16)
    # f32 biases bit-packed as bf16 pairs (little-endian), bitcast on device
    wpack[:, 240:246] = biases.view(np.uint16).view(BF16)

    qb = q.astype(BF16)
    in_maps = []
    for c in range(N_CORES):
        qc = qb[c * BPC:(c + 1) * BPC]
        # atom n = s*8192 + ch*1024 + j  ->  partition 12*ch+f, col s*1024+j
        Ac = np.ascontiguousarray(
            qc.reshape(SG, CHUNKS, SGW, F)
              .transpose(1, 3, 0, 2).reshape(CHUNKS * F, FREE))
        in_maps.append({"xin": Ac, "wpack": wpack})
    return in_maps, force_prev


def _host_gather(results):
    out = np.empty((B, 3), np.float32)
    for c in range(N_CORES):
        Oc = np.asarray(results[c]["out"]).astype(np.float32)    # [112, 1024]
        Oc = np.concatenate([Oc[0:48], Oc[64:112]])              # drop pad
        # row = 48*p + 24*sp + 3*ch + f; col j; n = (2p+sp)*8192 + ch*1024 + j
        oc = (Oc.reshape(2, 2, 8, 3, SGW)
                .transpose(0, 1, 2, 4, 3).reshape(BPC, 3))
        out[c * BPC:(c + 1) * BPC] = oc
    return out


_LAST_RES = None  # BassKernelResults of the most recent run (for test harness)


def kernel(x, W1, b1, W2, b2, Ws, bs, Wo, bo):
    global _LAST_RES
    from concourse.bass_utils import run_bass_kernel_spmd

    in_maps, force_prev = _host_prep(x, W1, b1, W2, b2, Ws, bs, Wo, bo)
    nc = _build_nc()
    res = run_bass_kernel_spmd(nc, in_maps, core_ids=list(range(N_CORES)))
    _LAST_RES = res
    out = _host_gather(res.results)
    return (out, force_prev)


# revision 22
# speedup vs baseline: 1.1589x; 1.0004x over previous
"""Trainium2 Bass kernel for nn_LIMADNN2_42013370090068 (dense_mlp).

Reference semantics: out depends only on x[:, 0, :] — the `state.add(...)`
neighbor loop in the torch module is not in-place, so the 65-neighbor
dimension is dead. force_prev = x[:, 0, 6:9] is a pure slice.

  q   = x[:, 0, :]                 # [B, 12]
  h   = relu(q @ W1 + b1)          # [B, 16]
  blk = relu(h @ W2 + b2)          # [B, 8]
  out = (blk @ Ws + bs) @ Wo + bo  # [B, 3]   (no relu between -> folded)

Device strategy (pure data parallel, 8 cores, batch-sharded):
  * Host slices q (12.6 MB of the 818 MB input), computes force_prev, and
    folds Ws/Wo into one [8,3] matrix. All matmul operands bf16 (fp32
    PSUM): 1 cyc/col on the PE, half the HBM bytes; end-to-end rel err
    ~7e-3 vs the 2e-2 gate.
  * Features-on-partitions, 8 batch-chunks block-diagonal per matmul,
    N=1024 moving columns (bf16 max) to minimize instruction count —
    per-matmul sequencer cost (ldweights+matmul ~340 ns) dominates over
    column streaming for this size.
  * L2 outputs of two consecutive supergroups land in one [128,1024]
    PSUM tile at partition offsets 0/64, halving relu2 columns; L3 uses
    16-chunk block-diag [128,48] per pair, pairs at offsets 0/64 of one
    [112,1024] PSUM tile (rows 48-63/112-127 dead), so two copy+bias ops
    and two DMAs drain the whole core's output.
  * PE HAM warm-up: dummy matmuls on the weight tile while input DMAs
    stream; ACT activation-table preload via a dummy relu at t=0.
  * f32 biases ride bit-packed in the bf16 weight tile (bitcast APs) and
    are folded into the PSUM->SBUF ops; out bias bso added on-device.
"""

import numpy as np
import ml_dtypes

B = 262144
F = 12
N_CORES = 8
BPC = B // N_CORES          # 32768 atoms per core
CHUNKS = 8                  # batch chunks packed on PE partitions (L1/L2)
SG = 4                      # supergroups per core
SGW = 1024                  # moving columns per supergroup matmul
FREE = SG * SGW             # 4096 input columns per core
WCOLS = 246                 # packed weight tensor columns (bf16; f32 biases
                            # bit-packed as bf16 pairs at cols 240-245)
N_WARMUP = 8                # PE warm-up matmuls (N=240 each)

BF16 = ml_dtypes.bfloat16


def _build_nc():
    import concourse.tile as tile
    from concourse import bacc, mybir

    f32 = mybir.dt.float32
    bf16 = mybir.dt.bfloat16

    nc = bacc.Bacc("TRN2", target_bir_lowering=False, debug=False,
                   num_devices=N_CORES)

    xin = nc.dram_tensor("xin", [CHUNKS * F, FREE], bf16, kind="ExternalInput")
    wpack = nc.dram_tensor("wpack", [128, WCOLS], bf16, kind="ExternalInput")
    # rows: 64*pair + 24*sp + 3*chunk + f; rows 48-63/112-127 dead padding
    out = nc.dram_tensor("out", [112, SGW], bf16, kind="ExternalOutput")

    Relu = mybir.ActivationFunctionType.Relu
    add, vmax = mybir.AluOpType.add, mybir.AluOpType.max

    def mm(ps_ap, lhsT_ap, rhs_ap):
        nc.tensor.matmul(ps_ap, lhsT_ap, rhs_ap, start=True, stop=True)

    with tile.TileContext(nc) as tc:
        with (
            tc.tile_pool(name="const", bufs=1) as cpool,
            tc.tile_pool(name="dmy", bufs=1) as dpool,
            tc.tile_pool(name="xt", bufs=4) as xpool,
            tc.tile_pool(name="h", bufs=2) as hpool,
            tc.tile_pool(name="blk", bufs=2) as bpool,
            tc.tile_pool(name="osb", bufs=2) as opool,
            tc.tile_pool(name="ps1", bufs=2, space="PSUM") as ps1pool,
            tc.tile_pool(name="ps2", bufs=1, space="PSUM") as ps2pool,
            tc.tile_pool(name="ps3", bufs=1, space="PSUM") as ps3pool,
        ):
            # ACT activation-table preload: dummy relu on a memset tile so
            # the auto-inserted LoadActFuncSet (~1.3 us) runs at t=0,
            # concurrent with the input DMAs, not before the first real relu.
            dmy = dpool.tile([1, 2], bf16)
            nc.vector.memset(dmy[:], 0.0)
            dmy2 = dpool.tile([1, 2], bf16)
            nc.scalar.activation(dmy2[:], dmy[:], Relu)

            # Weight DMA issued from the ACT sequencer (HWDGE) so its config
            # overlaps SP issuing the first input chunk.
            wsb = cpool.tile([128, WCOLS], bf16)
            nc.scalar.dma_start(wsb[:], wpack[:])
            w1 = wsb[0:96, 0:128]
            w2 = wsb[0:128, 128:192]
            w3 = wsb[0:128, 192:240]
            b1 = wsb[0:128, 240:242].bitcast(f32)
            b2 = wsb[0:128, 242:244].bitcast(f32)

            # No PE warm-up matmuls: the first real matmul lands after the
            # ~3.5 us input-DMA latency, which already satisfies the HAM
            # ramp window; issuing PE work earlier only resets the ramp.
            ps3 = ps3pool.tile([112, SGW], f32)

            ps2 = None
            for s in range(SG):
                xt = xpool.tile([CHUNKS * F, SGW], bf16)
                nc.sync.dma_start(xt[:], xin[:, SGW * s:SGW * (s + 1)])

                ps1 = ps1pool.tile([128, SGW], f32)
                mm(ps1[:], w1, xt[:])
                h = hpool.tile([128, SGW], bf16)
                nc.scalar.activation(h[:], ps1[:], Relu, bias=b1)

                sp = s % 2
                if sp == 0:
                    ps2 = ps2pool.tile([128, SGW], f32)
                mm(ps2[64 * sp:64 * sp + 64, :], w2, h[:])

                if sp == 1:
                    p = s // 2
                    blk = bpool.tile([128, SGW], bf16)
                    nc.vector.tensor_scalar(blk[:], ps2[:], b2, 0.0, add, vmax)
                    mm(ps3[64 * p:64 * p + 48, :], w3, blk[:])

            # Drain both pairs: two half-width copy+bias ops (DVE) and two
            # DMAs (ACT sequencer: SP stays input-only), second half
            # overlapping the first DMA.
            bso = wsb[0:112, 244:246].bitcast(f32)
            for hcol in range(2):
                cols = slice(hcol * (SGW // 2), (hcol + 1) * (SGW // 2))
                osb = opool.tile([112, SGW // 2], bf16)
                nc.vector.tensor_scalar(osb[:], ps3[0:112, cols], bso, None,
                                        add)
                nc.scalar.dma_start(out[:, cols], osb[:])

    nc.finalize()
    return nc


def _host_prep(x, W1, b1, W2, b2, Ws, bs, Wo, bo):
    x = np.asarray(x)
    W1 = np.asarray(W1, dtype=np.float32)
    b1 = np.asarray(b1, dtype=np.float32)
    W2 = np.asarray(W2, dtype=np.float32)
    b2 = np.asarray(b2, dtype=np.float32)
    Ws = np.asarray(Ws, dtype=np.float32)
    bs = np.asarray(bs, dtype=np.float32)
    Wo = np.asarray(Wo, dtype=np.float32)
    bo = np.asarray(bo, dtype=np.float32)

    q = np.ascontiguousarray(x[:, 0, :], dtype=np.float32)       # [B, 12]
    force_prev = np.ascontiguousarray(x[:, 0, 6:9], dtype=np.float32)

    # Fold the two linear layers that have no nonlinearity between them.
    Wso = (Ws.astype(np.float64) @ Wo.astype(np.float64)).astype(np.float32)
    bso = (bs.astype(np.float64) @ Wo.astype(np.float64)
           + bo.astype(np.float64)).astype(np.float32)

    wts = np.zeros((128, 240), np.float32)
    for c in range(CHUNKS):
        wts[c * 12:(c + 1) * 12, c * 16:(c + 1) * 16] = W1
        wts[c * 16:(c + 1) * 16, 128 + c * 8:128 + (c + 1) * 8] = W2
    for k in range(16):
        wts[k * 8:(k + 1) * 8, 192 + k * 3:192 + (k + 1) * 3] = Wso
    p = np.arange(128)
    biases = np.zeros((128, 3), np.float32)
    biases[:, 0] = b1[p % 16]
    biases[:, 1] = b2[p % 8]
    biases[0:48, 2] = bso[p[0:48] % 3]
    biases[64:112, 2] = bso[p[0:48] % 3]
    wpack = np.zeros((128, WCOLS), BF16)
    wpack[:, 0:240] = wts.astype(BF16)
    # f32 biases bit-packed as bf16 pairs (little-endian), bitcast on device
    wpack[:, 240:246] = biases.view(np.uint16).view(BF16)

    qb = q.astype(BF16)
    in_maps = []
    for c in range(N_CORES):
        qc = qb[c * BPC:(c + 1) * BPC]
        # atom n = s*8192 + ch*1024 + j  ->  partition 12*ch+f, col s*1024+j
        Ac = np.ascontiguousarray(
            qc.reshape(SG, CHUNKS, SGW, F)
              .transpose(1, 3, 0, 2).reshape(CHUNKS * F, FREE))
        in_maps.append({"xin": Ac, "wpack": wpack})
    return in_maps, force_prev


def _host_gather(results):
    out = np.empty((B, 3), np.float32)
    for c in range(N_CORES):
        Oc = np.asarray(results[c]["out"]).astype(np.float32)    # [112, 1024]
        Oc = np.concatenate([Oc[0:48], Oc[64:112]])              # drop pad
        # row = 48*p + 24*sp + 3*ch + f; col j; n = (2p+sp)*8192 + ch*1024 + j
        oc = (Oc.reshape(2, 2, 8, 3, SGW)
                .transpose(0, 1, 2, 4, 3).reshape(BPC, 3))
        out[c * BPC:(c + 1) * BPC] = oc
    return out


_LAST_RES = None  # BassKernelResults of the most recent run (for test harness)


def kernel(x, W1, b1, W2, b2, Ws, bs, Wo, bo):
    global _LAST_RES
    from concourse.bass_utils import run_bass_kernel_spmd

    in_maps, force_prev = _host_prep(x, W1, b1, W2, b2, Ws, bs, Wo, bo)
    nc = _build_nc()
    res = run_bass_kernel_spmd(nc, in_maps, core_ids=list(range(N_CORES)))
    _LAST_RES = res
    out = _host_gather(res.results)
    return (out, force_prev)


# revision 24
# speedup vs baseline: 1.3761x; 1.1875x over previous
"""Trainium2 Bass kernel for nn_LIMADNN2_42013370090068 (dense_mlp).

Reference semantics: out depends only on x[:, 0, :] — the `state.add(...)`
neighbor loop in the torch module is not in-place, so the 65-neighbor
dimension is dead. force_prev = x[:, 0, 6:9] is a pure slice.

  q   = x[:, 0, :]                 # [B, 12]
  h   = relu(q @ W1 + b1)          # [B, 16]
  blk = relu(h @ W2 + b2)          # [B, 8]
  out = (blk @ Ws + bs) @ Wo + bo  # [B, 3]   (no relu between -> folded)

Device strategy (pure data parallel, 8 cores, batch-sharded):
  * Host slices q (12.6 MB of the 818 MB input), computes force_prev, and
    folds Ws/Wo into one [8,3] matrix. All matmul operands bf16 (fp32
    PSUM): 1 cyc/col on the PE, half the HBM bytes; end-to-end rel err
    ~7e-3 vs the 2e-2 gate.
  * Features-on-partitions, 8 batch-chunks block-diagonal per matmul,
    N=1024 moving columns (bf16 max) to minimize instruction count —
    per-matmul sequencer cost (ldweights+matmul ~340 ns) dominates over
    column streaming for this size.
  * L2 outputs of two consecutive supergroups land in one [128,1024]
    PSUM tile at partition offsets 0/64, halving relu2 columns; L3 uses
    16-chunk block-diag [128,48] per pair, pairs at offsets 0/64 of one
    [112,1024] PSUM tile (rows 48-63/112-127 dead), so two copy+bias ops
    and two DMAs drain the whole core's output.
  * PE HAM warm-up: dummy matmuls on the weight tile while input DMAs
    stream; ACT activation-table preload via a dummy relu at t=0.
  * f32 biases ride bit-packed in the bf16 weight tile (bitcast APs) and
    are folded into the PSUM->SBUF ops; out bias bso added on-device.
"""

import numpy as np
import ml_dtypes

B = 262144
F = 12
N_CORES = 8
BPC = B // N_CORES          # 32768 atoms per core
CHUNKS = 8                  # batch chunks packed on PE partitions (L1/L2)
SG = 4                      # supergroups per core
SGW = 1024                  # moving columns per supergroup matmul
FREE = SG * SGW             # 4096 input columns per core
WCOLS = 246                 # packed weight tensor columns (bf16; f32 biases
                            # bit-packed as bf16 pairs at cols 240-245)
N_WARMUP = 8                # PE warm-up matmuls (N=240 each)

BF16 = ml_dtypes.bfloat16


def _build_nc():
    import concourse.tile as tile
    from concourse import bacc, mybir

    f32 = mybir.dt.float32
    bf16 = mybir.dt.bfloat16

    nc = bacc.Bacc("TRN2", target_bir_lowering=False, debug=False,
                   num_devices=N_CORES)

    xin = nc.dram_tensor("xin", [CHUNKS * F, FREE], bf16, kind="ExternalInput")
    wpack = nc.dram_tensor("wpack", [128, WCOLS], bf16, kind="ExternalInput")
    # rows: 64*pair + 24*sp + 3*chunk + f; rows 48-63/112-127 dead padding
    out = nc.dram_tensor("out", [112, SGW], bf16, kind="ExternalOutput")

    Relu = mybir.ActivationFunctionType.Relu
    add, vmax = mybir.AluOpType.add, mybir.AluOpType.max

    def mm(ps_ap, lhsT_ap, rhs_ap):
        nc.tensor.matmul(ps_ap, lhsT_ap, rhs_ap, start=True, stop=True)

    with tile.TileContext(nc) as tc:
        with (
            tc.tile_pool(name="const", bufs=1) as cpool,
            tc.tile_pool(name="dmy", bufs=1) as dpool,
            tc.tile_pool(name="xt", bufs=4) as xpool,
            tc.tile_pool(name="h", bufs=2) as hpool,
            tc.tile_pool(name="blk", bufs=2) as bpool,
            tc.tile_pool(name="osb", bufs=2) as opool,
            tc.tile_pool(name="ps1", bufs=2, space="PSUM") as ps1pool,
            tc.tile_pool(name="ps2", bufs=1, space="PSUM") as ps2pool,
            tc.tile_pool(name="ps3", bufs=1, space="PSUM") as ps3pool,
        ):
            # ACT activation-table preload: dummy relu on a memset tile so
            # the auto-inserted LoadActFuncSet (~1.3 us) runs at t=0,
            # concurrent with the input DMAs, not before the first real relu.
            dmy = dpool.tile([1, 2], bf16)
            nc.vector.memset(dmy[:], 0.0)
            dmy2 = dpool.tile([1, 2], bf16)
            nc.scalar.activation(dmy2[:], dmy[:], Relu)

            # Weight DMA on SP, ahead of the input chunks: delivering wsb
            # EARLY is counterproductive — PE work issued before the ~3 us
            # ramp window runs at reduced clock (cost model and HW HAM
            # agree on this), so wsb arriving ~3.7 us keeps every matmul,
            # warm-up included, at full speed.
            wsb = cpool.tile([128, WCOLS], bf16)
            nc.sync.dma_start(wsb[:], wpack[:])
            w1 = wsb[0:96, 0:128]
            w2 = wsb[0:128, 128:192]
            w3 = wsb[0:128, 192:240]
            b1 = wsb[0:128, 240:242].bitcast(f32)
            b2 = wsb[0:128, 242:244].bitcast(f32)

            # HAM warm-up matmuls (fill PE idle before the first input chunk
            # lands); they write scratch columns of ps3, overwritten later.
            ps3 = ps3pool.tile([112, SGW], f32)
            for _ in range(N_WARMUP):
                mm(ps3[:, 0:240], wsb[0:128, 0:112], wsb[0:128, 0:240])

            ps2 = None
            for s in range(SG):
                xt = xpool.tile([CHUNKS * F, SGW], bf16)
                nc.sync.dma_start(xt[:], xin[:, SGW * s:SGW * (s + 1)])

                ps1 = ps1pool.tile([128, SGW], f32)
                mm(ps1[:], w1, xt[:])
                h = hpool.tile([128, SGW], bf16)
                nc.scalar.activation(h[:], ps1[:], Relu, bias=b1)

                sp = s % 2
                if sp == 0:
                    ps2 = ps2pool.tile([128, SGW], f32)
                mm(ps2[64 * sp:64 * sp + 64, :], w2, h[:])

                if sp == 1:
                    p = s // 2
                    blk = bpool.tile([128, SGW], bf16)
                    nc.vector.tensor_scalar(blk[:], ps2[:], b2, 0.0, add, vmax)
                    mm(ps3[64 * p:64 * p + 48, :], w3, blk[:])

            # Drain both pairs: two half-width copy+bias ops (DVE) and two
            # DMAs (ACT sequencer: SP stays input-only), second half
            # overlapping the first DMA.
            bso = wsb[0:112, 244:246].bitcast(f32)
            for hcol in range(2):
                cols = slice(hcol * (SGW // 2), (hcol + 1) * (SGW // 2))
                osb = opool.tile([112, SGW // 2], bf16)
                nc.vector.tensor_scalar(osb[:], ps3[0:112, cols], bso, None,
                                        add)
                nc.scalar.dma_start(out[:, cols], osb[:])

    nc.finalize()
    return nc


def _host_prep(x, W1, b1, W2, b2, Ws, bs, Wo, bo):
    x = np.asarray(x)
    W1 = np.asarray(W1, dtype=np.float32)
    b1 = np.asarray(b1, dtype=np.float32)
    W2 = np.asarray(W2, dtype=np.float32)
    b2 = np.asarray(b2, dtype=np.float32)
    Ws = np.asarray(Ws, dtype=np.float32)
    bs = np.asarray(bs, dtype=np.float32)
    Wo = np.asarray(Wo, dtype=np.float32)
    bo = np.asarray(bo, dtype=np.float32)

    q = np.ascontiguousarray(x[:, 0, :], dtype=np.float32)       # [B, 12]
    force_prev = np.ascontiguousarray(x[:, 0, 6:9], dtype=np.float32)

    # Fold the two linear layers that have no nonlinearity between them.
    Wso = (Ws.astype(np.float64) @ Wo.astype(np.float64)).astype(np.float32)
    bso = (bs.astype(np.float64) @ Wo.astype(np.float64)
           + bo.astype(np.float64)).astype(np.float32)

    wts = np.zeros((128, 240), np.float32)
    for c in range(CHUNKS):
        wts[c * 12:(c + 1) * 12, c * 16:(c + 1) * 16] = W1
        wts[c * 16:(c + 1) * 16, 128 + c * 8:128 + (c + 1) * 8] = W2
    for k in range(16):
        wts[k * 8:(k + 1) * 8, 192 + k * 3:192 + (k + 1) * 3] = Wso
    p = np.arange(128)
    biases = np.zeros((128, 3), np.float32)
    biases[:, 0] = b1[p % 16]
    biases[:, 1] = b2[p % 8]
    biases[0:48, 2] = bso[p[0:48] % 3]
    biases[64:112, 2] = bso[p[0:48] % 3]
    wpack = np.zeros((128, WCOLS), BF16)
    wpack[:, 0:240] = wts.astype(BF16)
    # f32 biases bit-packed as bf16 pairs (little-endian), bitcast on device
    wpack[:, 240:246] = biases.view(np.uint16).view(BF16)

    qb = q.astype(BF16)
    in_maps = []
    for c in range(N_CORES):
        qc = qb[c * BPC:(c + 1) * BPC]
        # atom n = s*8192 + ch*1024 + j  ->  partition 12*ch+f, col s*1024+j
        Ac = np.ascontiguousarray(
            qc.reshape(SG, CHUNKS, SGW, F)
              .transpose(1, 3, 0, 2).reshape(CHUNKS * F, FREE))
        in_maps.append({"xin": Ac, "wpack": wpack})
    return in_maps, force_prev


def _host_gather(results):
    out = np.empty((B, 3), np.float32)
    for c in range(N_CORES):
        Oc = np.asarray(results[c]["out"]).astype(np.float32)    # [112, 1024]
        Oc = np.concatenate([Oc[0:48], Oc[64:112]])              # drop pad
        # row = 48*p + 24*sp + 3*ch + f; col j; n = (2p+sp)*8192 + ch*1024 + j
        oc = (Oc.reshape(2, 2, 8, 3, SGW)
                .transpose(0, 1, 2, 4, 3).reshape(BPC, 3))
        out[c * BPC:(c + 1) * BPC] = oc
    return out


_LAST_RES = None  # BassKernelResults of the most recent run (for test harness)


def kernel(x, W1, b1, W2, b2, Ws, bs, Wo, bo):
    global _LAST_RES
    from concourse.bass_utils import run_bass_kernel_spmd

    in_maps, force_prev = _host_prep(x, W1, b1, W2, b2, Ws, bs, Wo, bo)
    nc = _build_nc()
    res = run_bass_kernel_spmd(nc, in_maps, core_ids=list(range(N_CORES)))
    _LAST_RES = res
    out = _host_gather(res.results)
    return (out, force_prev)
